# revision 7
# baseline (speedup 1.0000x reference)
"""Multi-head attention (B=8, N=1024, DIM=768, H=12) on 8 Trainium2 NeuronCores.

Sharding: pure data-parallel over the batch dimension — core c computes batch
element c end-to-end (qkv projection, softmax attention, output projection).
No collectives needed.

Per-core kernel layout strategy (all fp32 storage, matmuls via float32r):
  stage 1: qkT = [Wq;Wk] @ x^T   -> SBUF [128, 12, 1024]   (rows (h,d), q then k)
           v   = x @ Wv^T        -> SBUF [128, 8, 12, 65]  (per j-tile, per head,
                                     64 v columns + a ones column for the
                                     softmax denominator)
  stage 2: per head h:
           S^T[j,i] = k_h^T q_h  (j on partitions -> softmax denom via matmul)
           P = exp(S*scale)      (ScalarE, no max subtraction: |S|<~8 so exp
                                  is safely in fp32 range)
           OT_aug[d,i] = sum_j v_aug[j,d] P[j,i]  (row 64 = sum_j P = denom l)
           r = 1/l; r broadcast across partitions via rank-1 matmul;
           OT_norm = OT * r fused into the PSUM->SBUF copy
  stage 3: y = OT^T @ WpT + bias  -> DRAM
"""

import os
import sys

for _p in ("/opt/trn_rl_repo",):
    if _p not in sys.path:
        sys.path.insert(0, _p)

import numpy as np

import concourse.bass as bass
import concourse.tile as tile
from concourse import bacc, mybir

B, N, DIM, H = 8, 1024, 768, 12
D = DIM // H  # 64
SCALE = D ** -0.5
P = 128
KT = DIM // P        # 6 contraction tiles over dim
NT = N // P          # 8 tiles over sequence
MT_QK = (2 * DIM) // P  # 12 output tiles for q,k
NCH = N // 512       # 2 moving-dim chunks of 512
FP = mybir.dt.float32
F32R = mybir.dt.float32r

USE_F32R = True
# matmul-input dtype: float32r is the PE's single-pass fp32 mode (1 cyc/row
# at moving>=256 vs 4 cyc/row for plain fp32). The BIR verifier requires
# every f32r matmul operand to be PRODUCED by a rounding instruction
# (DVE/ACT copy with f32r output), so DMA-loaded tensors go through a
# staging tile + rounding copy.
MMDT = F32R if USE_F32R else FP


def _chunks(total, size):
    return [(lo, min(lo + size, total)) for lo in range(0, total, size)]


def build_nc():
    nc = bacc.Bacc(None, target_bir_lowering=False)
    xT = nc.dram_tensor("xT", [DIM, N], FP, kind="ExternalInput")
    wqkT = nc.dram_tensor("wqkT", [DIM, 2 * DIM], FP, kind="ExternalInput")
    wvT = nc.dram_tensor("wvT", [DIM, DIM], FP, kind="ExternalInput")
    wpT = nc.dram_tensor("wpT", [DIM, DIM], FP, kind="ExternalInput")
    bias = nc.dram_tensor("bias", [1, DIM], FP, kind="ExternalInput")
    y = nc.dram_tensor("y", [N, DIM], FP, kind="ExternalOutput")

    with tile.TileContext(nc) as tc:
        with nc.allow_low_precision(
            reason="float32r matmul inputs (PE fast-path fp32)"
        ):
            _body(tc, xT, wqkT, wvT, wpT, bias, y)
    nc.compile()
    return nc


def _body(tc, xT, wqkT, wvT, wpT, bias, y):
    nc = tc.nc
    Exp = mybir.ActivationFunctionType.Exp

    with (
        tc.tile_pool(name="persist", bufs=1) as persist,
    ):
        qkT_sb = persist.tile([P, MT_QK, N], MMDT)      # 48 KB/part
        v_sb = persist.tile([P, NT, H, D + 1], MMDT)    # 25.4 KB/part
        oT_sb = persist.tile([P, KT, N], MMDT)          # 24 KB/part
        bias_sb = persist.tile([P, DIM], FP)            # 3 KB/part
        ones_sb = persist.tile([1, P], MMDT)

        nc.sync.dma_start(out=bias_sb, in_=bias[:].to_broadcast((P, DIM)))
        # memset can't write float32r; stage in fp32 then round-copy
        ones_stg = persist.tile([P, P], FP)
        nc.vector.memset(ones_stg, 1.0)
        nc.vector.tensor_copy(out=ones_sb, in_=ones_stg[0:1, :])
        nc.vector.tensor_copy(
            out=v_sb[:, :, :, D],
            in_=ones_stg[:, 0:NT * H].rearrange("p (j h) -> p j h", h=H),
        )

        # ---------------- stage 1: qkT / v projections ----------------
        with (
            tc.tile_pool(name="s1w", bufs=1) as s1w,
            tc.tile_pool(name="s1stg", bufs=4) as s1stg,
            tc.tile_pool(name="s1ps", bufs=2, space="PSUM") as s1ps,
        ):
            xT_sb = s1w.tile([P, KT, N], MMDT)          # 24 KB/part
            wqkT_sb = s1w.tile([P, KT, 2 * DIM], MMDT)  # 36 KB/part
            wvT_sb = s1w.tile([P, KT, DIM], MMDT)       # 18 KB/part

            xTr = xT[:].rearrange("(t p) n -> t p n", p=P)
            wqkr = wqkT[:].rearrange("(t p) m -> t p m", p=P)
            wvr = wvT[:].rearrange("(t p) m -> t p m", p=P)
            for t in range(KT):
                for dst, src, w in (
                    (xT_sb, xTr, N),
                    (wqkT_sb, wqkr, 2 * DIM),
                    (wvT_sb, wvr, DIM),
                ):
                    stg = s1stg.tile([P, 2 * DIM], FP, tag="stg")
                    nc.sync.dma_start(out=stg[:, :w], in_=src[t])
                    nc.vector.tensor_copy(out=dst[:, t], in_=stg[:, :w])

            # qkT[m*128:(m+1)*128, :] = sum_k wqk_chunk^T @ xT
            for m in range(MT_QK):
                ps = s1ps.tile([P, N], FP, tag="s1")
                for lo, hi in _chunks(N, 512):
                    for k in range(KT):
                        nc.tensor.matmul(
                            ps[:, lo:hi],
                            wqkT_sb[:, k, m * P:(m + 1) * P],
                            xT_sb[:, k, lo:hi],
                            start=(k == 0),
                            stop=(k == KT - 1),
                        )
                nc.vector.tensor_copy(out=qkT_sb[:, m], in_=ps)

            # v[j*128:(j+1)*128, :] = sum_k x_chunk^T... (natural orientation)
            for j in range(NT):
                ps = s1ps.tile([P, DIM], FP, tag="s1")
                for lo, hi in _chunks(DIM, 512):
                    for k in range(KT):
                        nc.tensor.matmul(
                            ps[:, lo:hi],
                            xT_sb[:, k, j * P:(j + 1) * P],
                            wvT_sb[:, k, lo:hi],
                            start=(k == 0),
                            stop=(k == KT - 1),
                        )
                nc.vector.tensor_copy(
                    out=v_sb[:, j, :, 0:D],
                    in_=ps.rearrange("p (h d) -> p h d", d=D),
                )

        # ---------------- stage 2: attention per head ----------------
        with (
            tc.tile_pool(name="expp", bufs=3) as expp,
            tc.tile_pool(name="rp", bufs=2) as rp,
            tc.tile_pool(name="stps", bufs=2, space="PSUM") as stps,
            tc.tile_pool(name="otps", bufs=2, space="PSUM") as otps,
        ):
            for h in range(H):
                ht, hp = divmod(h, 2)
                hp *= D
                qT = qkT_sb[hp:hp + D, ht]          # [64, 1024]
                kT = qkT_sb[hp:hp + D, 6 + ht]      # [64, 1024]
                ot = otps.tile([D + 1, N], FP, tag="ot")

                def issue_st(j):
                    st = stps.tile([P, N], FP, tag="st")
                    ex = expp.tile([P, N], MMDT, tag="exp")
                    for lo, hi in _chunks(N, 512):
                        nc.tensor.matmul(
                            st[:, lo:hi],
                            kT[:, j * P:(j + 1) * P],
                            qT[:, lo:hi],
                            start=True,
                            stop=True,
                        )
                    for lo, hi in _chunks(N, 512):
                        nc.scalar.activation(
                            out=ex[:, lo:hi], in_=st[:, lo:hi],
                            func=Exp, scale=float(SCALE),
                        )
                    return ex

                def issue_pv(j, ex):
                    for lo, hi in _chunks(N, 512):
                        nc.tensor.matmul(
                            ot[:, lo:hi],
                            v_sb[:, j, h, :],
                            ex[:, lo:hi],
                            start=(j == 0),
                            stop=(j == NT - 1),
                        )

                exps = [issue_st(0), issue_st(1)]
                for j in range(NT):
                    issue_pv(j, exps[j])
                    if j + 2 < NT:
                        exps.append(issue_st(j + 2))

                # softmax denominator -> reciprocal -> broadcast across parts
                r_row = rp.tile([1, N], MMDT, tag="rrow")
                nc.vector.reciprocal(out=r_row, in_=ot[D:D + 1, :])
                rb_ps = stps.tile([P, N], FP, tag="st")
                for lo, hi in _chunks(N, 512):
                    nc.tensor.matmul(
                        rb_ps[:, lo:hi],
                        ones_sb,
                        r_row[:, lo:hi],
                        start=True,
                        stop=True,
                    )
                rb_sb = rp.tile([P, N], FP, tag="rb")
                nc.vector.tensor_copy(out=rb_sb, in_=rb_ps)

                # normalized head output into oT_sb rows (h*64 .. h*64+64)
                nc.vector.tensor_tensor(
                    out=oT_sb[hp:hp + D, ht],
                    in0=ot[0:D],
                    in1=rb_sb[0:D],
                    op=mybir.AluOpType.mult,
                )

        # ---------------- stage 3: output projection ----------------
        with (
            tc.tile_pool(name="s3w", bufs=1) as s3w,
            tc.tile_pool(name="s3y", bufs=2) as s3y,
            tc.tile_pool(name="s3ps", bufs=2, space="PSUM") as s3ps,
        ):
            wpT_sb = s3w.tile([P, KT, DIM], MMDT)
            wpr = wpT[:].rearrange("(t p) m -> t p m", p=P)
            for t in range(KT):
                stg = s3w.tile([P, DIM], FP, tag="wpstg")
                nc.sync.dma_start(out=stg, in_=wpr[t])
                nc.vector.tensor_copy(out=wpT_sb[:, t], in_=stg)

            yr = y[:].rearrange("(i p) e -> i p e", p=P)
            for i in range(NT):
                ps = s3ps.tile([P, DIM], FP, tag="y")
                for lo, hi in _chunks(DIM, 512):
                    for t in range(KT):
                        nc.tensor.matmul(
                            ps[:, lo:hi],
                            oT_sb[:, t, i * P:(i + 1) * P],
                            wpT_sb[:, t, lo:hi],
                            start=(t == 0),
                            stop=(t == KT - 1),
                        )
                y_sb = s3y.tile([P, DIM], FP, tag="ysb")
                nc.vector.tensor_tensor(
                    out=y_sb, in0=ps, in1=bias_sb, op=mybir.AluOpType.add,
                )
                nc.sync.dma_start(out=yr[i], in_=y_sb)


def prep_inputs(x, w_qkv, w_proj, b_proj):
    x = np.asarray(x, dtype=np.float32)
    w_qkv = np.asarray(w_qkv, dtype=np.float32)
    w_proj = np.asarray(w_proj, dtype=np.float32)
    b_proj = np.asarray(b_proj, dtype=np.float32)

    w_r = w_qkv.reshape(H, D, 3, DIM)  # rows ordered (h, d, qkv)
    wq = w_r[:, :, 0, :].reshape(DIM, DIM)
    wk = w_r[:, :, 1, :].reshape(DIM, DIM)
    wv = w_r[:, :, 2, :].reshape(DIM, DIM)
    wqkT = np.ascontiguousarray(np.concatenate([wq, wk], 0).T)  # [768, 1536]
    wvT = np.ascontiguousarray(wv.T)                            # [768, 768]
    wpT = np.ascontiguousarray(w_proj.T)                        # [768, 768]
    xT = np.ascontiguousarray(x.transpose(0, 2, 1))             # [8, 768, 1024]
    bias = np.ascontiguousarray(b_proj.reshape(1, DIM))
    return xT, wqkT, wvT, wpT, bias


_NC = None
last_results = None


def get_nc():
    global _NC
    if _NC is None:
        _NC = build_nc()
    return _NC


def kernel(x, w_qkv, w_proj, b_proj):
    global last_results
    from concourse.bass_utils import run_bass_kernel_spmd

    nc = get_nc()
    xT, wqkT, wvT, wpT, bias = prep_inputs(x, w_qkv, w_proj, b_proj)
    in_maps = [
        {"xT": xT[c], "wqkT": wqkT, "wvT": wvT, "wpT": wpT, "bias": bias}
        for c in range(B)
    ]
    res = run_bass_kernel_spmd(nc, in_maps, core_ids=list(range(B)))
    last_results = res
    return np.stack([res.results[c]["y"] for c in range(B)], axis=0)


# revision 12
# speedup vs baseline: 1.3478x; 1.3478x over previous
"""Multi-head attention (B=8, N=1024, DIM=768, H=12) on 8 Trainium2 NeuronCores.

Sharding: pure data-parallel over the batch dimension — core c computes batch
element c end-to-end (qkv projection, softmax attention, output projection).
No collectives needed.

Per-core schedule (v2): stage-1 projection matmuls are interleaved INTO the
attention loop as PE filler so the tensor engine never idles while ScalarE
computes exp (an idle PE re-throttles to 1.2 GHz via HAM and doubles all
matmul times — measured on v1).

  prologue: DMA x, then per-pair slices of Wqk / Wv (f32r rounding copies)
  per head-pair t (heads 2t, 2t+1):
      qk pair-tile:  qkT[:, 2t], qkT[:, 2t+1] = [Wq_t; Wk_t] @ x^T
      v pair-slice:  v[:, :, 2t:2t+2] = x @ Wv_t^T   (+ ones column)
      per head: S^T[j,i] = k^T q (j on partitions), P = exp(S*scale) in one
      ACTIVATE per j-tile, OT_aug = v_aug^T... PV accumulates [d+1, i] with
      row 64 = softmax denominator l; these interleave with the NEXT pair's
      qk/v matmuls as PE filler.
      r-chain: l -> f32r row, rank-1 ones⊗l broadcast (PSUM, shares ST
      slots), reciprocal_approx_fast (128-lane), normalize fused into the
      PSUM->SBUF copy of OT.
  stage 3: y = OT^T @ WpT + bias
"""

import os
import sys

for _p in ("/opt/trn_rl_repo",):
    if _p not in sys.path:
        sys.path.insert(0, _p)

import numpy as np

import concourse.bass as bass
import concourse.tile as tile
from concourse import bacc, mybir

B, N, DIM, H = 8, 1024, 768, 12
D = DIM // H  # 64
SCALE = D ** -0.5
P = 128
KT = DIM // P        # 6 contraction tiles over dim
NT = N // P          # 8 tiles over sequence
NPAIR = H // 2       # 6 head pairs
FP = mybir.dt.float32
F32R = mybir.dt.float32r

USE_F32R = True
# float32r is the PE's single-pass fp32 mode (1 cyc/row at moving>=256 vs
# 4 cyc/row for plain fp32). Every f32r matmul operand must be PRODUCED by
# a rounding instruction (DVE/ACT output), so DMA-loaded tensors go through
# a staging tile + rounding copy.
MMDT = F32R if USE_F32R else FP


def _chunks(total, size):
    return [(lo, min(lo + size, total)) for lo in range(0, total, size)]


def build_nc():
    nc = bacc.Bacc(None, target_bir_lowering=False)
    xT = nc.dram_tensor("xT", [DIM, N], FP, kind="ExternalInput")
    # wqkT columns are pair-blocked: [q_t | k_t] of 128 cols each, t=0..5
    wqkT = nc.dram_tensor("wqkT", [DIM, 2 * DIM], FP, kind="ExternalInput")
    wvT = nc.dram_tensor("wvT", [DIM, DIM], FP, kind="ExternalInput")
    wpT = nc.dram_tensor("wpT", [DIM, DIM], FP, kind="ExternalInput")
    bias = nc.dram_tensor("bias", [1, DIM], FP, kind="ExternalInput")
    y = nc.dram_tensor("y", [N, DIM], FP, kind="ExternalOutput")

    with tile.TileContext(nc) as tc:
        with nc.allow_low_precision(
            reason="float32r matmul inputs (PE fast-path fp32)"
        ):
            _body(tc, xT, wqkT, wvT, wpT, bias, y)
    nc.compile()
    return nc


def _body(tc, xT, wqkT, wvT, wpT, bias, y):
    nc = tc.nc
    Exp = mybir.ActivationFunctionType.Exp
    Mult = mybir.AluOpType.mult
    Add = mybir.AluOpType.add

    from contextlib import ExitStack
    with tc.tile_pool(name="persist", bufs=1) as persist:
      with ExitStack() as s12:
        s1w = s12.enter_context(tc.tile_pool(name="s1w", bufs=1))
        s1stg = s12.enter_context(tc.tile_pool(name="s1stg", bufs=4))
        expp = s12.enter_context(tc.tile_pool(name="expp", bufs=3))
        rp = s12.enter_context(tc.tile_pool(name="rp", bufs=1))
        s1ps = s12.enter_context(tc.tile_pool(name="s1ps", bufs=1, space="PSUM"))
        stps = s12.enter_context(tc.tile_pool(name="stps", bufs=2, space="PSUM"))
        otps = s12.enter_context(tc.tile_pool(name="otps", bufs=1, space="PSUM"))
        # qkT_sb tile index 2t = q of pair t, 2t+1 = k of pair t; rows (h%2,d)
        qkT_sb = persist.tile([P, 2 * KT, N], MMDT)     # 48 KB/part
        v_sb = persist.tile([P, NT, H, D + 1], MMDT)    # 25.4 KB/part
        oT_sb = persist.tile([P, KT, N], MMDT)          # 24 KB/part
        bias_sb = persist.tile([P, DIM], FP)            # 3 KB/part
        ones_sb = persist.tile([1, P], MMDT)

        nc.sync.dma_start(out=bias_sb, in_=bias[:].to_broadcast((P, DIM)))
        ones_stg = persist.tile([P, P], FP)
        nc.vector.memset(ones_stg, 1.0)
        nc.vector.tensor_copy(out=ones_sb, in_=ones_stg[0:1, :])
        nc.vector.tensor_copy(
            out=v_sb[:, :, :, D],
            in_=ones_stg[:, 0:NT * H].rearrange("p (j h) -> p j h", h=H),
        )

        xT_sb = s1w.tile([P, KT, N], MMDT)              # 24 KB/part
        wqkT_sb = s1w.tile([P, KT, 2 * DIM], MMDT)      # 36 KB/part
        wvT_sb = s1w.tile([P, KT, DIM], MMDT)           # 18 KB/part

        xTr = xT[:].rearrange("(t p) n -> t p n", p=P)
        wqkr = wqkT[:].rearrange("(t p) m -> t p m", p=P)
        wvr = wvT[:].rearrange("(t p) m -> t p m", p=P)

        # DMA + f32r rounding copy, ordered so pair-0 work unblocks first:
        # x (all), wqk pair-0 slices, wv pair-0, remaining pairs, wp later.
        def load(dst, src_r, k, lo, hi, tag):
            stg = s1stg.tile([P, 512], FP, tag="stg")
            nc.sync.dma_start(out=stg[:, 0:hi - lo], in_=src_r[k][:, lo:hi])
            nc.vector.tensor_copy(out=dst[:, k, lo:hi], in_=stg[:, 0:hi - lo])

        for k in range(KT):
            for lo, hi in _chunks(N, 512):
                load(xT_sb, xTr, k, lo, hi, "x")
        for k in range(KT):
            load(wqkT_sb, wqkr, k, 0, 256, "wqk0")
        for k in range(KT):
            load(wvT_sb, wvr, k, 0, P, "wv0")
        for t in range(1, NPAIR):
            for k in range(KT):
                load(wqkT_sb, wqkr, k, t * 256, (t + 1) * 256, "wqk")
            for k in range(KT):
                load(wvT_sb, wvr, k, t * P, (t + 1) * P, "wv")

        # ---- PE work generators (filler units: ~1-2 us of matmuls each) ----
        def gen_qk(t):
            """qk pair-tile t -> qkT_sb[:, 2t] (q) and [:, 2t+1] (k).
            Yields after each ~1.3us matmul group (filler granularity)."""
            for which in range(2):
                ps = s1ps.tile([P, N], FP, tag="s1")
                for lo, hi in _chunks(N, 512):
                    for k in range(KT):
                        nc.tensor.matmul(
                            ps[:, lo:hi],
                            wqkT_sb[:, k, t * 256 + which * P:
                                    t * 256 + (which + 1) * P],
                            xT_sb[:, k, lo:hi],
                            start=(k == 0),
                            stop=(k == KT - 1),
                        )
                    yield
                nc.vector.tensor_copy(out=qkT_sb[:, 2 * t + which], in_=ps)

        def gen_v(t):
            """v pair-slice t -> v_sb[:, :, 2t:2t+2, 0:D]."""
            ps = s1ps.tile([P, N], FP, tag="s1")
            for j in range(NT):
                for k in range(KT):
                    nc.tensor.matmul(
                        ps[:, j * P:(j + 1) * P],
                        xT_sb[:, k, j * P:(j + 1) * P],
                        wvT_sb[:, k, t * P:(t + 1) * P],
                        start=(k == 0),
                        stop=(k == KT - 1),
                    )
                yield
            nc.vector.tensor_copy(
                out=v_sb[:, :, 2 * t:2 * t + 2, 0:D],
                in_=ps.rearrange("p (j g d) -> p j g d", g=2, d=D),
            )

        def head_attn(h, filler):
            """Attention for head h; pulls PE filler between steps."""
            t, hp = divmod(h, 2)
            hp *= D
            qT = qkT_sb[hp:hp + D, 2 * t]
            kT = qkT_sb[hp:hp + D, 2 * t + 1]
            ot = otps.tile([D + 1, N], FP, tag="ot")

            def issue_st(j):
                st = stps.tile([P, N], FP, tag="st")
                ex = expp.tile([P, N], MMDT, tag="exp")
                for lo, hi in _chunks(N, 512):
                    nc.tensor.matmul(
                        st[:, lo:hi],
                        kT[:, j * P:(j + 1) * P],
                        qT[:, lo:hi],
                        start=True,
                        stop=True,
                    )
                nc.scalar.activation(
                    out=ex, in_=st, func=Exp, scale=float(SCALE),
                )
                return ex

            def issue_pv(j, ex):
                for lo, hi in _chunks(N, 512):
                    nc.tensor.matmul(
                        ot[:, lo:hi],
                        v_sb[:, j, h, :],
                        ex[:, lo:hi],
                        start=(j == 0),
                        stop=(j == NT - 1),
                    )

            def pull():
                try:
                    next(filler)
                except StopIteration:
                    pass

            exps = [issue_st(0), issue_st(1)]
            for j in range(NT):
                issue_pv(j, exps[j])
                pull()
                if j + 2 < NT:
                    exps.append(issue_st(j + 2))

            # r-chain: l -> f32r SBUF row -> rank-1 broadcast (PSUM, shares
            # the ST slots) -> approx reciprocal -> normalize fused into the
            # OT evacuation copy.
            l_row = rp.tile([1, N], MMDT, tag="lrow")
            nc.vector.tensor_copy(out=l_row, in_=ot[D:D + 1, :])
            pull()
            lb = stps.tile([P, N], FP, tag="st")
            for lo, hi in _chunks(N, 512):
                nc.tensor.matmul(
                    lb[:, lo:hi], ones_sb, l_row[:, lo:hi],
                    start=True, stop=True,
                )
            rb_sb = rp.tile([P, N], FP, tag="rb")
            nc.vector.reciprocal_approx_fast(out=rb_sb, in_=lb)
            nc.vector.tensor_tensor(
                out=oT_sb[hp:hp + D, t], in0=ot[0:D], in1=rb_sb[0:D], op=Mult,
            )

        # ---- interleaved pair loop ----
        def filler_for_pair(t):
            """PE filler consumed during pair t's attention: next pair's
            qk + v projections."""
            if t + 1 < NPAIR:
                yield from gen_qk(t + 1)
                yield from gen_v(t + 1)

        for _ in gen_qk(0):
            pass
        for _ in gen_v(0):
            pass
        for t in range(NPAIR):
            f = filler_for_pair(t)
            head_attn(2 * t, f)
            head_attn(2 * t + 1, f)
            for _ in f:
                pass

      # ---------------- stage 3: output projection ----------------
      with (
            tc.tile_pool(name="s3w", bufs=1) as s3w,
            tc.tile_pool(name="s3y", bufs=2) as s3y,
            tc.tile_pool(name="s3ps", bufs=2, space="PSUM") as s3ps,
      ):
            wpT_sb = s3w.tile([P, KT, DIM], MMDT)
            wpr = wpT[:].rearrange("(t p) m -> t p m", p=P)
            for k in range(KT):
                stg = s3w.tile([P, DIM], FP, tag="wpstg")
                nc.sync.dma_start(out=stg, in_=wpr[k])
                nc.vector.tensor_copy(out=wpT_sb[:, k], in_=stg)

            yr = y[:].rearrange("(i p) e -> i p e", p=P)
            for i in range(NT):
                ps = s3ps.tile([P, DIM], FP, tag="y")
                for lo, hi in _chunks(DIM, 512):
                    for k in range(KT):
                        nc.tensor.matmul(
                            ps[:, lo:hi],
                            oT_sb[:, k, i * P:(i + 1) * P],
                            wpT_sb[:, k, lo:hi],
                            start=(k == 0),
                            stop=(k == KT - 1),
                        )
                y_sb = s3y.tile([P, DIM], FP, tag="ysb")
                nc.vector.tensor_tensor(
                    out=y_sb, in0=ps, in1=bias_sb, op=Add,
                )
                nc.sync.dma_start(out=yr[i], in_=y_sb)


def prep_inputs(x, w_qkv, w_proj, b_proj):
    x = np.asarray(x, dtype=np.float32)
    w_qkv = np.asarray(w_qkv, dtype=np.float32)
    w_proj = np.asarray(w_proj, dtype=np.float32)
    b_proj = np.asarray(b_proj, dtype=np.float32)

    w_r = w_qkv.reshape(H, D, 3, DIM)  # rows ordered (h, d, qkv)
    wq = w_r[:, :, 0, :].reshape(DIM, DIM)  # rows (h, d)
    wk = w_r[:, :, 1, :].reshape(DIM, DIM)
    wv = w_r[:, :, 2, :].reshape(DIM, DIM)
    # pair-blocked qk: columns [q_t (128) | k_t (128)] for t = 0..5
    wqk_pairs = np.empty((2 * DIM, DIM), dtype=np.float32)
    for t in range(NPAIR):
        wqk_pairs[t * 256:t * 256 + P] = wq[t * P:(t + 1) * P]
        wqk_pairs[t * 256 + P:(t + 1) * 256] = wk[t * P:(t + 1) * P]
    wqkT = np.ascontiguousarray(wqk_pairs.T)                    # [768, 1536]
    wvT = np.ascontiguousarray(wv.T)                            # [768, 768]
    wpT = np.ascontiguousarray(w_proj.T)                        # [768, 768]
    xT = np.ascontiguousarray(x.transpose(0, 2, 1))             # [8, 768, 1024]
    bias = np.ascontiguousarray(b_proj.reshape(1, DIM))
    return xT, wqkT, wvT, wpT, bias


_NC = None
last_results = None


def get_nc():
    global _NC
    if _NC is None:
        _NC = build_nc()
    return _NC


def kernel(x, w_qkv, w_proj, b_proj):
    global last_results
    from concourse.bass_utils import run_bass_kernel_spmd

    nc = get_nc()
    xT, wqkT, wvT, wpT, bias = prep_inputs(x, w_qkv, w_proj, b_proj)
    in_maps = [
        {"xT": xT[c], "wqkT": wqkT, "wvT": wvT, "wpT": wpT, "bias": bias}
        for c in range(B)
    ]
    res = run_bass_kernel_spmd(nc, in_maps, core_ids=list(range(B)))
    last_results = res
    return np.stack([res.results[c]["y"] for c in range(B)], axis=0)


# revision 13
# speedup vs baseline: 1.5050x; 1.1167x over previous
"""Multi-head attention (B=8, N=1024, DIM=768, H=12) on 8 Trainium2 NeuronCores.

Sharding: pure data-parallel over the batch dimension — core c computes batch
element c end-to-end (qkv projection, softmax attention, output projection).
No collectives needed.

Numerics: matmul inputs in bf16 (x, weights, q/k, v, exp(P)) with fp32 PSUM
accumulation; softmax denominator, reciprocal, normalization and bias in
fp32. bf16 weight loads are FWL-accelerated and overlap matmuls via the PE
background weight buffer (fp32/f32r self-loading matmuls serialize a 213ns
LDWEIGHTS with every matmul - measured; that was the v2 bottleneck).

Schedule: stage-1 projection matmuls are interleaved INTO the attention loop
as PE filler so the tensor engine never idles while ScalarE computes exp (an
idle PE re-throttles to 1.2 GHz via HAM and doubles all matmul times).

  per head-pair t (heads 2t, 2t+1):
      qk pair-tile:  qkT[:, 2t], qkT[:, 2t+1] = [Wq_t; Wk_t] @ x^T
      v pair-slice:  v[:, :, 2t:2t+2] = x @ Wv_t^T   (+ ones column)
      per head: S^T[j,i] = k^T q (j on partitions), P = exp(S*scale) in one
      ACTIVATE per j-tile; PV accumulates OT_aug [d+1, i] with row 64 =
      softmax denominator l (ones column of v_aug).
      r-chain: l row -> SBUF (fp32) -> DRAM bounce -> broadcast DMA to all
      128 partitions -> reciprocal_approx_fast -> normalize fused into the
      PSUM->SBUF copy of OT (bf16 out).
  stage 3: y = OT^T @ WpT + bias (fp32)
"""

import os
import sys

for _p in ("/opt/trn_rl_repo",):
    if _p not in sys.path:
        sys.path.insert(0, _p)

import ml_dtypes
import numpy as np

import concourse.bass as bass
import concourse.tile as tile
from concourse import bacc, mybir

B, N, DIM, H = 8, 1024, 768, 12
D = DIM // H  # 64
SCALE = D ** -0.5
P = 128
KT = DIM // P        # 6 contraction tiles over dim
NT = N // P          # 8 tiles over sequence
NPAIR = H // 2       # 6 head pairs
FP = mybir.dt.float32
BF = mybir.dt.bfloat16
MMDT = BF
NP_MMDT = ml_dtypes.bfloat16


def _chunks(total, size):
    return [(lo, min(lo + size, total)) for lo in range(0, total, size)]


def build_nc():
    nc = bacc.Bacc(None, target_bir_lowering=False)
    xT = nc.dram_tensor("xT", [DIM, N], MMDT, kind="ExternalInput")
    # wqkT columns are pair-blocked: [q_t | k_t] of 128 cols each, t=0..5
    wqkT = nc.dram_tensor("wqkT", [DIM, 2 * DIM], MMDT, kind="ExternalInput")
    wvT = nc.dram_tensor("wvT", [DIM, DIM], MMDT, kind="ExternalInput")
    wpT = nc.dram_tensor("wpT", [DIM, DIM], MMDT, kind="ExternalInput")
    bias = nc.dram_tensor("bias", [1, DIM], FP, kind="ExternalInput")
    y = nc.dram_tensor("y", [N, DIM], FP, kind="ExternalOutput")

    with tile.TileContext(nc) as tc:
        with nc.allow_low_precision(reason="bf16 matmul inputs"):
            _body(tc, xT, wqkT, wvT, wpT, bias, y)
    nc.compile()
    return nc


def _body(tc, xT, wqkT, wvT, wpT, bias, y):
    nc = tc.nc
    Exp = mybir.ActivationFunctionType.Exp
    Mult = mybir.AluOpType.mult
    Add = mybir.AluOpType.add

    from contextlib import ExitStack
    with tc.tile_pool(name="persist", bufs=1) as persist:
      with ExitStack() as s12:
        s1w = s12.enter_context(tc.tile_pool(name="s1w", bufs=1))
        expp = s12.enter_context(tc.tile_pool(name="expp", bufs=3))
        rp = s12.enter_context(tc.tile_pool(name="rp", bufs=2))
        dramp = s12.enter_context(tc.tile_pool(name="dramp", bufs=2, space="DRAM"))
        s1ps = s12.enter_context(tc.tile_pool(name="s1ps", bufs=1, space="PSUM"))
        stps = s12.enter_context(tc.tile_pool(name="stps", bufs=2, space="PSUM"))
        otps = s12.enter_context(tc.tile_pool(name="otps", bufs=1, space="PSUM"))

        # qkT_sb tile index 2t = q of pair t, 2t+1 = k of pair t; rows (h%2,d)
        qkT_sb = persist.tile([P, 2 * KT, N], MMDT)     # 24 KB/part
        v_sb = persist.tile([P, NT, H, D + 1], MMDT)    # 12.7 KB/part
        oT_sb = persist.tile([P, KT, N], MMDT)          # 12 KB/part
        bias_sb = persist.tile([P, DIM], FP)            # 3 KB/part
        nc.sync.dma_start(out=bias_sb, in_=bias[:].to_broadcast((P, DIM)))
        nc.vector.memset(v_sb[:, :, :, D], 1.0)

        xT_sb = s1w.tile([P, KT, N], MMDT)              # 12 KB/part
        wqkT_sb = s1w.tile([P, KT, 2 * DIM], MMDT)      # 18 KB/part
        wvT_sb = s1w.tile([P, KT, DIM], MMDT)           # 9 KB/part

        xTr = xT[:].rearrange("(t p) n -> t p n", p=P)
        wqkr = wqkT[:].rearrange("(t p) m -> t p m", p=P)
        wvr = wvT[:].rearrange("(t p) m -> t p m", p=P)

        # DMA order: x first (everything needs it), then per-pair slices of
        # wqk / wv so pair-0 unblocks earliest.
        for k in range(KT):
            nc.sync.dma_start(out=xT_sb[:, k], in_=xTr[k])
        for k in range(KT):
            nc.sync.dma_start(out=wqkT_sb[:, k, 0:256], in_=wqkr[k][:, 0:256])
        for k in range(KT):
            nc.sync.dma_start(out=wvT_sb[:, k, 0:P], in_=wvr[k][:, 0:P])
        for t in range(1, NPAIR):
            for k in range(KT):
                nc.sync.dma_start(
                    out=wqkT_sb[:, k, t * 256:(t + 1) * 256],
                    in_=wqkr[k][:, t * 256:(t + 1) * 256],
                )
            for k in range(KT):
                nc.sync.dma_start(
                    out=wvT_sb[:, k, t * P:(t + 1) * P],
                    in_=wvr[k][:, t * P:(t + 1) * P],
                )

        # ---- PE work generators (filler units of ~0.5-1.3us of matmuls) ----
        def gen_qk(t):
            """qk pair-tile t -> qkT_sb[:, 2t] (q) and [:, 2t+1] (k)."""
            for which in range(2):
                ps = s1ps.tile([P, N], FP, tag="s1")
                for lo, hi in _chunks(N, 512):
                    for k in range(KT):
                        nc.tensor.matmul(
                            ps[:, lo:hi],
                            wqkT_sb[:, k, t * 256 + which * P:
                                    t * 256 + (which + 1) * P],
                            xT_sb[:, k, lo:hi],
                            start=(k == 0),
                            stop=(k == KT - 1),
                        )
                    yield
                nc.vector.tensor_copy(out=qkT_sb[:, 2 * t + which], in_=ps)

        def gen_v(t):
            """v pair-slice t -> v_sb[:, :, 2t:2t+2, 0:D]."""
            ps = s1ps.tile([P, N], FP, tag="s1")
            for j in range(NT):
                for k in range(KT):
                    nc.tensor.matmul(
                        ps[:, j * P:(j + 1) * P],
                        xT_sb[:, k, j * P:(j + 1) * P],
                        wvT_sb[:, k, t * P:(t + 1) * P],
                        start=(k == 0),
                        stop=(k == KT - 1),
                    )
                yield
            nc.vector.tensor_copy(
                out=v_sb[:, :, 2 * t:2 * t + 2, 0:D],
                in_=ps.rearrange("p (j g d) -> p j g d", g=2, d=D),
            )

        def head_attn(h, filler):
            """Attention for head h; pulls PE filler between steps."""
            t, hp = divmod(h, 2)
            hp *= D
            qT = qkT_sb[hp:hp + D, 2 * t]
            kT = qkT_sb[hp:hp + D, 2 * t + 1]
            ot = otps.tile([D + 1, N], FP, tag="ot")

            def issue_st(j):
                st = stps.tile([P, N], FP, tag="st")
                ex = expp.tile([P, N], MMDT, tag="exp")
                for lo, hi in _chunks(N, 512):
                    nc.tensor.matmul(
                        st[:, lo:hi],
                        kT[:, j * P:(j + 1) * P],
                        qT[:, lo:hi],
                        start=True,
                        stop=True,
                    )
                nc.scalar.activation(
                    out=ex, in_=st, func=Exp, scale=float(SCALE),
                )
                return ex

            def issue_pv(j, ex):
                for lo, hi in _chunks(N, 512):
                    nc.tensor.matmul(
                        ot[:, lo:hi],
                        v_sb[:, j, h, :],
                        ex[:, lo:hi],
                        start=(j == 0),
                        stop=(j == NT - 1),
                    )

            def pull():
                try:
                    next(filler)
                except StopIteration:
                    pass

            exps = [issue_st(0), issue_st(1)]
            for j in range(NT):
                issue_pv(j, exps[j])
                pull()
                if j + 2 < NT:
                    exps.append(issue_st(j + 2))

            # r-chain (all fp32): l row -> SBUF -> DRAM -> broadcast back ->
            # approx reciprocal -> normalize fused into the OT evacuation.
            l_row = rp.tile([1, N], FP, tag="lrow")
            nc.vector.tensor_copy(out=l_row, in_=ot[D:D + 1, :])
            l_dram = dramp.tile([1, N], FP, tag="ldram")
            nc.sync.dma_start(out=l_dram, in_=l_row)
            lb_sb = rp.tile([P, N], FP, tag="lb")
            nc.sync.dma_start(out=lb_sb, in_=l_dram[:].to_broadcast((P, N)))
            rb_sb = rp.tile([P, N], FP, tag="rb")
            nc.vector.reciprocal_approx_fast(out=rb_sb, in_=lb_sb)
            nc.vector.tensor_tensor(
                out=oT_sb[hp:hp + D, t], in0=ot[0:D], in1=rb_sb[0:D], op=Mult,
            )

        # ---- interleaved pair loop ----
        def filler_for_pair(t):
            if t + 1 < NPAIR:
                yield from gen_qk(t + 1)
                yield from gen_v(t + 1)

        for _ in gen_qk(0):
            pass
        for _ in gen_v(0):
            pass
        for t in range(NPAIR):
            f = filler_for_pair(t)
            head_attn(2 * t, f)
            head_attn(2 * t + 1, f)
            for _ in f:
                pass

      # ---------------- stage 3: output projection ----------------
      with (
            tc.tile_pool(name="s3w", bufs=1) as s3w,
            tc.tile_pool(name="s3y", bufs=2) as s3y,
            tc.tile_pool(name="s3ps", bufs=2, space="PSUM") as s3ps,
      ):
            wpT_sb = s3w.tile([P, KT, DIM], MMDT)
            wpr = wpT[:].rearrange("(t p) m -> t p m", p=P)
            for k in range(KT):
                nc.sync.dma_start(out=wpT_sb[:, k], in_=wpr[k])

            yr = y[:].rearrange("(i p) e -> i p e", p=P)
            for i in range(NT):
                ps = s3ps.tile([P, DIM], FP, tag="y")
                for lo, hi in _chunks(DIM, 512):
                    for k in range(KT):
                        nc.tensor.matmul(
                            ps[:, lo:hi],
                            oT_sb[:, k, i * P:(i + 1) * P],
                            wpT_sb[:, k, lo:hi],
                            start=(k == 0),
                            stop=(k == KT - 1),
                        )
                y_sb = s3y.tile([P, DIM], FP, tag="ysb")
                nc.vector.tensor_tensor(
                    out=y_sb, in0=ps, in1=bias_sb, op=Add,
                )
                nc.sync.dma_start(out=yr[i], in_=y_sb)


def prep_inputs(x, w_qkv, w_proj, b_proj):
    x = np.asarray(x, dtype=np.float32)
    w_qkv = np.asarray(w_qkv, dtype=np.float32)
    w_proj = np.asarray(w_proj, dtype=np.float32)
    b_proj = np.asarray(b_proj, dtype=np.float32)

    w_r = w_qkv.reshape(H, D, 3, DIM)  # rows ordered (h, d, qkv)
    wq = w_r[:, :, 0, :].reshape(DIM, DIM)  # rows (h, d)
    wk = w_r[:, :, 1, :].reshape(DIM, DIM)
    wv = w_r[:, :, 2, :].reshape(DIM, DIM)
    # pair-blocked qk: columns [q_t (128) | k_t (128)] for t = 0..5
    wqk_pairs = np.empty((2 * DIM, DIM), dtype=np.float32)
    for t in range(NPAIR):
        wqk_pairs[t * 256:t * 256 + P] = wq[t * P:(t + 1) * P]
        wqk_pairs[t * 256 + P:(t + 1) * 256] = wk[t * P:(t + 1) * P]
    wqkT = np.ascontiguousarray(wqk_pairs.T).astype(NP_MMDT)    # [768, 1536]
    wvT = np.ascontiguousarray(wv.T).astype(NP_MMDT)            # [768, 768]
    wpT = np.ascontiguousarray(w_proj.T).astype(NP_MMDT)        # [768, 768]
    xT = np.ascontiguousarray(x.transpose(0, 2, 1)).astype(NP_MMDT)
    bias = np.ascontiguousarray(b_proj.reshape(1, DIM))
    return xT, wqkT, wvT, wpT, bias


_NC = None
last_results = None


def get_nc():
    global _NC
    if _NC is None:
        _NC = build_nc()
    return _NC


def kernel(x, w_qkv, w_proj, b_proj):
    global last_results
    from concourse.bass_utils import run_bass_kernel_spmd

    nc = get_nc()
    xT, wqkT, wvT, wpT, bias = prep_inputs(x, w_qkv, w_proj, b_proj)
    in_maps = [
        {"xT": xT[c], "wqkT": wqkT, "wvT": wvT, "wpT": wpT, "bias": bias}
        for c in range(B)
    ]
    res = run_bass_kernel_spmd(nc, in_maps, core_ids=list(range(B)))
    last_results = res
    return np.stack([res.results[c]["y"] for c in range(B)], axis=0)


# revision 15
# speedup vs baseline: 1.9695x; 1.3086x over previous
"""Multi-head attention (B=8, N=1024, DIM=768, H=12) on 8 Trainium2 NeuronCores.

Sharding: pure data-parallel over the batch dimension — core c computes batch
element c end-to-end (qkv projection, softmax attention, output projection).
No collectives needed.

Numerics: matmul inputs in bf16 (x, weights, q/k, v, exp(P)) with fp32 PSUM
accumulation; softmax denominator, reciprocal, normalization and bias in
fp32. bf16 weight loads are FWL-accelerated and overlap matmuls via the PE
background weight buffer (fp32/f32r self-loading matmuls serialize a 213ns
LDWEIGHTS with every matmul - measured; that was the v2 bottleneck).

Schedule: stage-1 projection matmuls are interleaved INTO the attention loop
as PE filler so the tensor engine never idles while ScalarE computes exp (an
idle PE re-throttles to 1.2 GHz via HAM and doubles all matmul times).

  per head-pair t (heads 2t, 2t+1):
      qk pair-tile:  qkT[:, 2t], qkT[:, 2t+1] = [Wq_t; Wk_t] @ x^T
      v pair-slice:  v[:, :, 2t:2t+2] = x @ Wv_t^T   (+ ones column)
      per head: S^T[j,i] = k^T q (j on partitions), P = exp(S*scale) in one
      ACTIVATE per j-tile; PV accumulates OT_aug [d+1, i] with row 64 =
      softmax denominator l (ones column of v_aug).
      r-chain: l row -> SBUF (fp32) -> DRAM bounce -> broadcast DMA to all
      128 partitions -> reciprocal_approx_fast -> normalize fused into the
      PSUM->SBUF copy of OT (bf16 out).
  stage 3: y = OT^T @ WpT + bias (fp32)
"""

import os
import sys

for _p in ("/opt/trn_rl_repo",):
    if _p not in sys.path:
        sys.path.insert(0, _p)

import ml_dtypes
import numpy as np

import concourse.bass as bass
import concourse.tile as tile
from concourse import bacc, mybir

B, N, DIM, H = 8, 1024, 768, 12
D = DIM // H  # 64
SCALE = D ** -0.5
P = 128
KT = DIM // P        # 6 contraction tiles over dim
NT = N // P          # 8 tiles over sequence
NPAIR = H // 2       # 6 head pairs
FP = mybir.dt.float32
BF = mybir.dt.bfloat16
MMDT = BF
NP_MMDT = ml_dtypes.bfloat16


def _chunks(total, size):
    return [(lo, min(lo + size, total)) for lo in range(0, total, size)]


def build_nc():
    nc = bacc.Bacc(None, target_bir_lowering=False)
    xT = nc.dram_tensor("xT", [DIM, N], MMDT, kind="ExternalInput")
    # wqkT columns are pair-blocked: [q_t | k_t] of 128 cols each, t=0..5
    wqkT = nc.dram_tensor("wqkT", [DIM, 2 * DIM], MMDT, kind="ExternalInput")
    wvT = nc.dram_tensor("wvT", [DIM, DIM], MMDT, kind="ExternalInput")
    wpT = nc.dram_tensor("wpT", [DIM, DIM], MMDT, kind="ExternalInput")
    bias = nc.dram_tensor("bias", [1, DIM], FP, kind="ExternalInput")
    y = nc.dram_tensor("y", [N, DIM], FP, kind="ExternalOutput")

    with tile.TileContext(nc) as tc:
        with nc.allow_low_precision(reason="bf16 matmul inputs"):
            _body(tc, xT, wqkT, wvT, wpT, bias, y)
    nc.compile()
    return nc


def _body(tc, xT, wqkT, wvT, wpT, bias, y):
    nc = tc.nc
    Exp = mybir.ActivationFunctionType.Exp
    Mult = mybir.AluOpType.mult
    Add = mybir.AluOpType.add

    from contextlib import ExitStack
    with tc.tile_pool(name="persist", bufs=1) as persist:
      with ExitStack() as s12:
        s1w = s12.enter_context(tc.tile_pool(name="s1w", bufs=1))
        expp = s12.enter_context(tc.tile_pool(name="expp", bufs=3))
        rp = s12.enter_context(tc.tile_pool(name="rp", bufs=2))
        s1ps = s12.enter_context(tc.tile_pool(name="s1ps", bufs=1, space="PSUM"))
        stps = s12.enter_context(tc.tile_pool(name="stps", bufs=2, space="PSUM"))
        otps = s12.enter_context(tc.tile_pool(name="otps", bufs=3, space="PSUM"))

        # qkT_sb tile index 2t = q of pair t, 2t+1 = k of pair t; rows (h%2,d)
        qkT_sb = persist.tile([P, 2 * KT, N], MMDT)     # 24 KB/part
        v_sb = persist.tile([P, NT, H, D + 1], MMDT)    # 12.7 KB/part
        oT_sb = persist.tile([P, KT, N], MMDT)          # 12 KB/part
        bias_sb = persist.tile([P, DIM], FP)            # 3 KB/part
        ones_f32r = persist.tile([1, P], mybir.dt.float32r)
        ones_stg = persist.tile([1, P], FP)
        nc.sync.dma_start(out=bias_sb, in_=bias[:].to_broadcast((P, DIM)))
        nc.vector.memset(v_sb[:, :, :, D], 1.0)
        nc.vector.memset(ones_stg, 1.0)
        nc.vector.tensor_copy(out=ones_f32r, in_=ones_stg)

        xT_sb = s1w.tile([P, KT, N], MMDT)              # 12 KB/part
        wqkT_sb = s1w.tile([P, KT, 2 * DIM], MMDT)      # 18 KB/part
        wvT_sb = s1w.tile([P, KT, DIM], MMDT)           # 9 KB/part

        xTr = xT[:].rearrange("(t p) n -> t p n", p=P)
        wqkr = wqkT[:].rearrange("(t p) m -> t p m", p=P)
        wvr = wvT[:].rearrange("(t p) m -> t p m", p=P)

        # DMA order: x first (everything needs it), then per-pair slices of
        # wqk / wv so pair-0 unblocks earliest.
        for k in range(KT):
            nc.sync.dma_start(out=xT_sb[:, k], in_=xTr[k])
        for k in range(KT):
            nc.sync.dma_start(out=wqkT_sb[:, k, 0:256], in_=wqkr[k][:, 0:256])
        for k in range(KT):
            nc.sync.dma_start(out=wvT_sb[:, k, 0:P], in_=wvr[k][:, 0:P])
        for t in range(1, NPAIR):
            for k in range(KT):
                nc.sync.dma_start(
                    out=wqkT_sb[:, k, t * 256:(t + 1) * 256],
                    in_=wqkr[k][:, t * 256:(t + 1) * 256],
                )
            for k in range(KT):
                nc.sync.dma_start(
                    out=wvT_sb[:, k, t * P:(t + 1) * P],
                    in_=wvr[k][:, t * P:(t + 1) * P],
                )

        # ---- PE work generators (filler units of ~0.5-1.3us of matmuls) ----
        def gen_qk(t):
            """qk pair-tile t -> qkT_sb[:, 2t] (q) and [:, 2t+1] (k)."""
            for which in range(2):
                for lo, hi in _chunks(N, 512):
                    ps = s1ps.tile([P, 512], FP, tag="s1")
                    for k in range(KT):
                        nc.tensor.matmul(
                            ps,
                            wqkT_sb[:, k, t * 256 + which * P:
                                    t * 256 + (which + 1) * P],
                            xT_sb[:, k, lo:hi],
                            start=(k == 0),
                            stop=(k == KT - 1),
                        )
                    nc.vector.tensor_copy(
                        out=qkT_sb[:, 2 * t + which, lo:hi], in_=ps)
                    yield

        def gen_v(t):
            """v pair-slice t -> v_sb[:, :, 2t:2t+2, 0:D]."""
            for half in range(2):
                ps = s1ps.tile([P, 512], FP, tag="s1")
                for jj in range(4):
                    j = half * 4 + jj
                    for k in range(KT):
                        nc.tensor.matmul(
                            ps[:, jj * P:(jj + 1) * P],
                            xT_sb[:, k, j * P:(j + 1) * P],
                            wvT_sb[:, k, t * P:(t + 1) * P],
                            start=(k == 0),
                            stop=(k == KT - 1),
                        )
                    yield
                nc.vector.tensor_copy(
                    out=v_sb[:, half * 4:(half + 1) * 4, 2 * t:2 * t + 2, 0:D],
                    in_=ps.rearrange("p (j g d) -> p j g d", g=2, d=D),
                )

        def head_attn(h, filler):
            """Attention for head h; pulls PE filler between steps."""
            t, hp = divmod(h, 2)
            hp *= D
            qT = qkT_sb[hp:hp + D, 2 * t]
            kT = qkT_sb[hp:hp + D, 2 * t + 1]
            # two 1-bank OT chunks (i cols 0:512 / 512:1024); a 3-slot pool
            # lets the next head's PV start while this head's r-chain runs
            ota = otps.tile([D + 1, 512], FP, tag="ot")
            otb = otps.tile([D + 1, 512], FP, tag="ot")
            ots = (ota, otb)

            def issue_st(j):
                st = stps.tile([P, N], FP, tag="st")
                ex = expp.tile([P, N], MMDT, tag="exp")
                for lo, hi in _chunks(N, 512):
                    nc.tensor.matmul(
                        st[:, lo:hi],
                        kT[:, j * P:(j + 1) * P],
                        qT[:, lo:hi],
                        start=True,
                        stop=True,
                    )
                nc.scalar.activation(
                    out=ex, in_=st, func=Exp, scale=float(SCALE),
                )
                return ex

            def issue_pv(j, ex):
                for c, (lo, hi) in enumerate(_chunks(N, 512)):
                    nc.tensor.matmul(
                        ots[c],
                        v_sb[:, j, h, :],
                        ex[:, lo:hi],
                        start=(j == 0),
                        stop=(j == NT - 1),
                    )

            def pull():
                try:
                    next(filler)
                except StopIteration:
                    pass

            exps = [issue_st(0), issue_st(1)]
            for j in range(NT):
                issue_pv(j, exps[j])
                pull()
                if j + 2 < NT:
                    exps.append(issue_st(j + 2))

            # r-chain: l rows (f32r) -> rank-1 ones⊗l broadcast into a PSUM
            # slot shared with the ST pool -> approx reciprocal (fp32) ->
            # normalize fused into the OT evacuation (bf16 out).
            la = rp.tile([1, 512], mybir.dt.float32r, tag="lrowa")
            lb_r = rp.tile([1, 512], mybir.dt.float32r, tag="lrowb")
            nc.vector.tensor_copy(out=la, in_=ota[D:D + 1, :])
            nc.vector.tensor_copy(out=lb_r, in_=otb[D:D + 1, :])
            pull()
            lbc = stps.tile([P, N], FP, tag="st")
            nc.tensor.matmul(lbc[:, 0:512], ones_f32r, la,
                             start=True, stop=True)
            nc.tensor.matmul(lbc[:, 512:N], ones_f32r, lb_r,
                             start=True, stop=True)
            rb_sb = rp.tile([P, N], FP, tag="rb")
            nc.vector.reciprocal_approx_fast(out=rb_sb, in_=lbc)
            nc.vector.tensor_tensor(
                out=oT_sb[hp:hp + D, t, 0:512], in0=ota[0:D],
                in1=rb_sb[0:D, 0:512], op=Mult,
            )
            nc.vector.tensor_tensor(
                out=oT_sb[hp:hp + D, t, 512:N], in0=otb[0:D],
                in1=rb_sb[0:D, 512:N], op=Mult,
            )

        # ---- interleaved pair loop ----
        def filler_for_pair(t):
            # ~16 units per pair, spread over the 18+ pulls of two heads by
            # inserting pacing skips
            if t + 1 < NPAIR:
                def units():
                    yield from gen_qk(t + 1)
                    yield from gen_v(t + 1)
                for i, u in enumerate(units()):
                    yield u
                    if i % 4 == 3:
                        yield None  # pacing skip

        for _ in gen_qk(0):
            pass
        for _ in gen_v(0):
            pass
        for t in range(NPAIR):
            f = filler_for_pair(t)
            head_attn(2 * t, f)
            head_attn(2 * t + 1, f)
            for _ in f:
                pass

      # ---------------- stage 3: output projection ----------------
      with (
            tc.tile_pool(name="s3w", bufs=1) as s3w,
            tc.tile_pool(name="s3y", bufs=2) as s3y,
            tc.tile_pool(name="s3ps", bufs=2, space="PSUM") as s3ps,
      ):
            wpT_sb = s3w.tile([P, KT, DIM], MMDT)
            wpr = wpT[:].rearrange("(t p) m -> t p m", p=P)
            for k in range(KT):
                nc.sync.dma_start(out=wpT_sb[:, k], in_=wpr[k])

            yr = y[:].rearrange("(i p) e -> i p e", p=P)
            for i in range(NT):
                ps = s3ps.tile([P, DIM], FP, tag="y")
                for lo, hi in _chunks(DIM, 512):
                    for k in range(KT):
                        nc.tensor.matmul(
                            ps[:, lo:hi],
                            oT_sb[:, k, i * P:(i + 1) * P],
                            wpT_sb[:, k, lo:hi],
                            start=(k == 0),
                            stop=(k == KT - 1),
                        )
                y_sb = s3y.tile([P, DIM], FP, tag="ysb")
                nc.vector.tensor_tensor(
                    out=y_sb, in0=ps, in1=bias_sb, op=Add,
                )
                nc.sync.dma_start(out=yr[i], in_=y_sb)


def prep_inputs(x, w_qkv, w_proj, b_proj):
    x = np.asarray(x, dtype=np.float32)
    w_qkv = np.asarray(w_qkv, dtype=np.float32)
    w_proj = np.asarray(w_proj, dtype=np.float32)
    b_proj = np.asarray(b_proj, dtype=np.float32)

    w_r = w_qkv.reshape(H, D, 3, DIM)  # rows ordered (h, d, qkv)
    wq = w_r[:, :, 0, :].reshape(DIM, DIM)  # rows (h, d)
    wk = w_r[:, :, 1, :].reshape(DIM, DIM)
    wv = w_r[:, :, 2, :].reshape(DIM, DIM)
    # pair-blocked qk: columns [q_t (128) | k_t (128)] for t = 0..5
    wqk_pairs = np.empty((2 * DIM, DIM), dtype=np.float32)
    for t in range(NPAIR):
        wqk_pairs[t * 256:t * 256 + P] = wq[t * P:(t + 1) * P]
        wqk_pairs[t * 256 + P:(t + 1) * 256] = wk[t * P:(t + 1) * P]
    wqkT = np.ascontiguousarray(wqk_pairs.T).astype(NP_MMDT)    # [768, 1536]
    wvT = np.ascontiguousarray(wv.T).astype(NP_MMDT)            # [768, 768]
    wpT = np.ascontiguousarray(w_proj.T).astype(NP_MMDT)        # [768, 768]
    xT = np.ascontiguousarray(x.transpose(0, 2, 1)).astype(NP_MMDT)
    bias = np.ascontiguousarray(b_proj.reshape(1, DIM))
    return xT, wqkT, wvT, wpT, bias


_NC = None
last_results = None


def get_nc():
    global _NC
    if _NC is None:
        _NC = build_nc()
    return _NC


def kernel(x, w_qkv, w_proj, b_proj):
    global last_results
    from concourse.bass_utils import run_bass_kernel_spmd

    nc = get_nc()
    xT, wqkT, wvT, wpT, bias = prep_inputs(x, w_qkv, w_proj, b_proj)
    in_maps = [
        {"xT": xT[c], "wqkT": wqkT, "wvT": wvT, "wpT": wpT, "bias": bias}
        for c in range(B)
    ]
    res = run_bass_kernel_spmd(nc, in_maps, core_ids=list(range(B)))
    last_results = res
    return np.stack([res.results[c]["y"] for c in range(B)], axis=0)


# revision 16
# speedup vs baseline: 1.9848x; 1.0078x over previous
"""Multi-head attention (B=8, N=1024, DIM=768, H=12) on 8 Trainium2 NeuronCores.

Sharding: pure data-parallel over the batch dimension — core c computes batch
element c end-to-end (qkv projection, softmax attention, output projection).
No collectives needed.

Numerics: matmul inputs in bf16 (x, weights, q/k, v, exp(P)) with fp32 PSUM
accumulation; softmax denominator, reciprocal, normalization and bias in
fp32. bf16 weight loads are FWL-accelerated and overlap matmuls via the PE
background weight buffer (fp32/f32r self-loading matmuls serialize a 213ns
LDWEIGHTS with every matmul - measured; that was the v2 bottleneck).

Schedule: stage-1 projection matmuls are interleaved INTO the attention loop
as PE filler so the tensor engine never idles while ScalarE computes exp (an
idle PE re-throttles to 1.2 GHz via HAM and doubles all matmul times).

  per head-pair t (heads 2t, 2t+1):
      qk pair-tile:  qkT[:, 2t], qkT[:, 2t+1] = [Wq_t; Wk_t] @ x^T
      v pair-slice:  v[:, :, 2t:2t+2] = x @ Wv_t^T   (+ ones column)
      per head: S^T[j,i] = k^T q (j on partitions), P = exp(S*scale) in one
      ACTIVATE per j-tile; PV accumulates OT_aug [d+1, i] with row 64 =
      softmax denominator l (ones column of v_aug).
      r-chain: l row -> SBUF (fp32) -> DRAM bounce -> broadcast DMA to all
      128 partitions -> reciprocal_approx_fast -> normalize fused into the
      PSUM->SBUF copy of OT (bf16 out).
  stage 3: y = OT^T @ WpT + bias (fp32)
"""

import os
import sys

for _p in ("/opt/trn_rl_repo",):
    if _p not in sys.path:
        sys.path.insert(0, _p)

import ml_dtypes
import numpy as np

import concourse.bass as bass
import concourse.tile as tile
from concourse import bacc, mybir

B, N, DIM, H = 8, 1024, 768, 12
D = DIM // H  # 64
SCALE = D ** -0.5
P = 128
KT = DIM // P        # 6 contraction tiles over dim
NT = N // P          # 8 tiles over sequence
NPAIR = H // 2       # 6 head pairs
FP = mybir.dt.float32
BF = mybir.dt.bfloat16
MMDT = BF
NP_MMDT = ml_dtypes.bfloat16


def _chunks(total, size):
    return [(lo, min(lo + size, total)) for lo in range(0, total, size)]


def build_nc():
    nc = bacc.Bacc(None, target_bir_lowering=False)
    xT = nc.dram_tensor("xT", [DIM, N], MMDT, kind="ExternalInput")
    # wqkT columns are pair-blocked: [q_t | k_t] of 128 cols each, t=0..5
    wqkT = nc.dram_tensor("wqkT", [DIM, 2 * DIM], MMDT, kind="ExternalInput")
    wvT = nc.dram_tensor("wvT", [DIM, DIM], MMDT, kind="ExternalInput")
    wpT = nc.dram_tensor("wpT", [DIM, DIM], MMDT, kind="ExternalInput")
    bias = nc.dram_tensor("bias", [1, DIM], FP, kind="ExternalInput")
    y = nc.dram_tensor("y", [N, DIM], FP, kind="ExternalOutput")

    with tile.TileContext(nc) as tc:
        with nc.allow_low_precision(reason="bf16 matmul inputs"):
            _body(tc, xT, wqkT, wvT, wpT, bias, y)
    nc.compile()
    return nc


def _body(tc, xT, wqkT, wvT, wpT, bias, y):
    nc = tc.nc
    Exp = mybir.ActivationFunctionType.Exp
    Mult = mybir.AluOpType.mult
    Add = mybir.AluOpType.add

    from contextlib import ExitStack
    with tc.tile_pool(name="persist", bufs=1) as persist:
      with ExitStack() as s12:
        s1w = s12.enter_context(tc.tile_pool(name="s1w", bufs=1))
        expp = s12.enter_context(tc.tile_pool(name="expp", bufs=3))
        rp = s12.enter_context(tc.tile_pool(name="rp", bufs=2))
        s1ps = s12.enter_context(tc.tile_pool(name="s1ps", bufs=1, space="PSUM"))
        stps = s12.enter_context(tc.tile_pool(name="stps", bufs=2, space="PSUM"))
        otps = s12.enter_context(tc.tile_pool(name="otps", bufs=3, space="PSUM"))

        # qkT_sb tile index 2t = q of pair t, 2t+1 = k of pair t; rows (h%2,d)
        qkT_sb = persist.tile([P, 2 * KT, N], MMDT)     # 24 KB/part
        v_sb = persist.tile([P, NT, H, D + 1], MMDT)    # 12.7 KB/part
        oT_sb = persist.tile([P, KT, N], MMDT)          # 12 KB/part
        bias_sb = persist.tile([P, DIM], FP)            # 3 KB/part
        ones_f32r = persist.tile([1, P], mybir.dt.float32r)
        ones_stg = persist.tile([1, P], FP)
        nc.sync.dma_start(out=bias_sb, in_=bias[:].to_broadcast((P, DIM)))
        nc.vector.memset(v_sb[:, :, :, D], 1.0)
        nc.vector.memset(ones_stg, 1.0)
        nc.vector.tensor_copy(out=ones_f32r, in_=ones_stg)

        xT_sb = s1w.tile([P, KT, N], MMDT)              # 12 KB/part
        wqkT_sb = s1w.tile([P, KT, 2 * DIM], MMDT)      # 18 KB/part
        wvT_sb = s1w.tile([P, KT, DIM], MMDT)           # 9 KB/part

        xTr = xT[:].rearrange("(t p) n -> t p n", p=P)
        wqkr = wqkT[:].rearrange("(t p) m -> t p m", p=P)
        wvr = wvT[:].rearrange("(t p) m -> t p m", p=P)

        # DMA order: x first (everything needs it), then per-pair slices of
        # wqk / wv so pair-0 unblocks earliest.
        for k in range(KT):
            nc.sync.dma_start(out=xT_sb[:, k], in_=xTr[k])
        for k in range(KT):
            nc.sync.dma_start(out=wqkT_sb[:, k, 0:256], in_=wqkr[k][:, 0:256])
        for k in range(KT):
            nc.sync.dma_start(out=wvT_sb[:, k, 0:P], in_=wvr[k][:, 0:P])
        for t in range(1, NPAIR):
            for k in range(KT):
                nc.sync.dma_start(
                    out=wqkT_sb[:, k, t * 256:(t + 1) * 256],
                    in_=wqkr[k][:, t * 256:(t + 1) * 256],
                )
            for k in range(KT):
                nc.sync.dma_start(
                    out=wvT_sb[:, k, t * P:(t + 1) * P],
                    in_=wvr[k][:, t * P:(t + 1) * P],
                )

        # ---- PE work generators (filler units of ~0.5-1.3us of matmuls) ----
        def gen_qk(t):
            """qk pair-tile t -> qkT_sb[:, 2t] (q) and [:, 2t+1] (k)."""
            for which in range(2):
                for lo, hi in _chunks(N, 512):
                    ps = s1ps.tile([P, 512], FP, tag="s1")
                    for k in range(KT):
                        nc.tensor.matmul(
                            ps,
                            wqkT_sb[:, k, t * 256 + which * P:
                                    t * 256 + (which + 1) * P],
                            xT_sb[:, k, lo:hi],
                            start=(k == 0),
                            stop=(k == KT - 1),
                        )
                    nc.vector.tensor_copy(
                        out=qkT_sb[:, 2 * t + which, lo:hi], in_=ps)
                    yield

        def gen_v(t):
            """v pair-slice t -> v_sb[:, :, 2t:2t+2, 0:D]."""
            for half in range(2):
                ps = s1ps.tile([P, 512], FP, tag="s1")
                for jj in range(4):
                    j = half * 4 + jj
                    for k in range(KT):
                        nc.tensor.matmul(
                            ps[:, jj * P:(jj + 1) * P],
                            xT_sb[:, k, j * P:(j + 1) * P],
                            wvT_sb[:, k, t * P:(t + 1) * P],
                            start=(k == 0),
                            stop=(k == KT - 1),
                        )
                    yield
                nc.vector.tensor_copy(
                    out=v_sb[:, half * 4:(half + 1) * 4, 2 * t:2 * t + 2, 0:D],
                    in_=ps.rearrange("p (j g d) -> p j g d", g=2, d=D),
                )

        def head_attn(h, filler, pending_rchain):
            """Attention for head h; pulls PE filler between steps.
            Issues its first two STs BEFORE running the previous head's
            r-chain (so ScalarE never starves at head boundaries), and
            returns its own r-chain as a closure for the next head."""
            t, hp = divmod(h, 2)
            hp *= D
            qT = qkT_sb[hp:hp + D, 2 * t]
            kT = qkT_sb[hp:hp + D, 2 * t + 1]
            # two 1-bank OT chunks (i cols 0:512 / 512:1024); a 3-slot pool
            # lets the next head's PV start while this head's r-chain runs
            ota = otps.tile([D + 1, 512], FP, tag="ot")
            otb = otps.tile([D + 1, 512], FP, tag="ot")
            ots = (ota, otb)

            def issue_st(j):
                st = stps.tile([P, N], FP, tag="st")
                ex = expp.tile([P, N], MMDT, tag="exp")
                for lo, hi in _chunks(N, 512):
                    nc.tensor.matmul(
                        st[:, lo:hi],
                        kT[:, j * P:(j + 1) * P],
                        qT[:, lo:hi],
                        start=True,
                        stop=True,
                    )
                nc.scalar.activation(
                    out=ex, in_=st, func=Exp, scale=float(SCALE),
                )
                return ex

            def issue_pv(j, ex):
                for c, (lo, hi) in enumerate(_chunks(N, 512)):
                    nc.tensor.matmul(
                        ots[c],
                        v_sb[:, j, h, :],
                        ex[:, lo:hi],
                        start=(j == 0),
                        stop=(j == NT - 1),
                    )

            def pull():
                try:
                    next(filler)
                except StopIteration:
                    pass

            exps = [issue_st(0), issue_st(1)]
            if pending_rchain is not None:
                pending_rchain()
            for j in range(NT):
                issue_pv(j, exps[j])
                pull()
                if j + 2 < NT:
                    exps.append(issue_st(j + 2))

            def rchain():
                # l rows (f32r) -> rank-1 ones⊗l broadcast into a PSUM slot
                # shared with the ST pool -> approx reciprocal (fp32) ->
                # normalize fused into the OT evacuation (bf16 out).
                la = rp.tile([1, 512], mybir.dt.float32r, tag="lrowa")
                lb_r = rp.tile([1, 512], mybir.dt.float32r, tag="lrowb")
                nc.vector.tensor_copy(out=la, in_=ota[D:D + 1, :])
                nc.vector.tensor_copy(out=lb_r, in_=otb[D:D + 1, :])
                pull()
                lbc = stps.tile([P, N], FP, tag="st")
                nc.tensor.matmul(lbc[:, 0:512], ones_f32r, la,
                                 start=True, stop=True)
                nc.tensor.matmul(lbc[:, 512:N], ones_f32r, lb_r,
                                 start=True, stop=True)
                rb_sb = rp.tile([P, N], FP, tag="rb")
                nc.vector.reciprocal_approx_fast(out=rb_sb, in_=lbc)
                nc.vector.tensor_tensor(
                    out=oT_sb[hp:hp + D, t, 0:512], in0=ota[0:D],
                    in1=rb_sb[0:D, 0:512], op=Mult,
                )
                nc.vector.tensor_tensor(
                    out=oT_sb[hp:hp + D, t, 512:N], in0=otb[0:D],
                    in1=rb_sb[0:D, 512:N], op=Mult,
                )

            return rchain

        # ---- interleaved pair loop ----
        def filler_for_pair(t):
            # ~16 units per pair, spread over the 18+ pulls of two heads by
            # inserting pacing skips
            if t + 1 < NPAIR:
                def units():
                    yield from gen_qk(t + 1)
                    yield from gen_v(t + 1)
                for i, u in enumerate(units()):
                    yield u
                    if i % 4 == 3:
                        yield None  # pacing skip

        for _ in gen_qk(0):
            pass
        for _ in gen_v(0):
            pass
        pending = None
        for t in range(NPAIR):
            f = filler_for_pair(t)
            pending = head_attn(2 * t, f, pending)
            pending = head_attn(2 * t + 1, f, pending)
            for _ in f:
                pass
        pending()

      # ---------------- stage 3: output projection ----------------
      with (
            tc.tile_pool(name="s3w", bufs=1) as s3w,
            tc.tile_pool(name="s3y", bufs=2) as s3y,
            tc.tile_pool(name="s3ps", bufs=2, space="PSUM") as s3ps,
      ):
            wpT_sb = s3w.tile([P, KT, DIM], MMDT)
            wpr = wpT[:].rearrange("(t p) m -> t p m", p=P)
            for k in range(KT):
                nc.sync.dma_start(out=wpT_sb[:, k], in_=wpr[k])

            yr = y[:].rearrange("(i p) e -> i p e", p=P)
            for i in range(NT):
                ps = s3ps.tile([P, DIM], FP, tag="y")
                for lo, hi in _chunks(DIM, 512):
                    for k in range(KT):
                        nc.tensor.matmul(
                            ps[:, lo:hi],
                            oT_sb[:, k, i * P:(i + 1) * P],
                            wpT_sb[:, k, lo:hi],
                            start=(k == 0),
                            stop=(k == KT - 1),
                        )
                y_sb = s3y.tile([P, DIM], FP, tag="ysb")
                nc.vector.tensor_tensor(
                    out=y_sb, in0=ps, in1=bias_sb, op=Add,
                )
                nc.sync.dma_start(out=yr[i], in_=y_sb)


def prep_inputs(x, w_qkv, w_proj, b_proj):
    x = np.asarray(x, dtype=np.float32)
    w_qkv = np.asarray(w_qkv, dtype=np.float32)
    w_proj = np.asarray(w_proj, dtype=np.float32)
    b_proj = np.asarray(b_proj, dtype=np.float32)

    w_r = w_qkv.reshape(H, D, 3, DIM)  # rows ordered (h, d, qkv)
    wq = w_r[:, :, 0, :].reshape(DIM, DIM)  # rows (h, d)
    wk = w_r[:, :, 1, :].reshape(DIM, DIM)
    wv = w_r[:, :, 2, :].reshape(DIM, DIM)
    # pair-blocked qk: columns [q_t (128) | k_t (128)] for t = 0..5
    wqk_pairs = np.empty((2 * DIM, DIM), dtype=np.float32)
    for t in range(NPAIR):
        wqk_pairs[t * 256:t * 256 + P] = wq[t * P:(t + 1) * P]
        wqk_pairs[t * 256 + P:(t + 1) * 256] = wk[t * P:(t + 1) * P]
    wqkT = np.ascontiguousarray(wqk_pairs.T).astype(NP_MMDT)    # [768, 1536]
    wvT = np.ascontiguousarray(wv.T).astype(NP_MMDT)            # [768, 768]
    wpT = np.ascontiguousarray(w_proj.T).astype(NP_MMDT)        # [768, 768]
    xT = np.ascontiguousarray(x.transpose(0, 2, 1)).astype(NP_MMDT)
    bias = np.ascontiguousarray(b_proj.reshape(1, DIM))
    return xT, wqkT, wvT, wpT, bias


_NC = None
last_results = None


def get_nc():
    global _NC
    if _NC is None:
        _NC = build_nc()
    return _NC


def kernel(x, w_qkv, w_proj, b_proj):
    global last_results
    from concourse.bass_utils import run_bass_kernel_spmd

    nc = get_nc()
    xT, wqkT, wvT, wpT, bias = prep_inputs(x, w_qkv, w_proj, b_proj)
    in_maps = [
        {"xT": xT[c], "wqkT": wqkT, "wvT": wvT, "wpT": wpT, "bias": bias}
        for c in range(B)
    ]
    res = run_bass_kernel_spmd(nc, in_maps, core_ids=list(range(B)))
    last_results = res
    return np.stack([res.results[c]["y"] for c in range(B)], axis=0)
